# revision 3
# baseline (speedup 1.0000x reference)
"""Object-condensation loss on 8 Trainium2 NeuronCores (Bass/Tile SPMD).

Hits are sharded 6250/core and laid out f-SORTED on each core so that
per-partition rows hold f-adjacent hits.  The dense [hits x 512] hinge
phase runs on PE (f32r matmuls, one PSUM bank per 128-hit group) with
fused one-pass consumers split across ACT (3-bank Relu+scale+accum
tiles) and DVE (1-bank min*scale+accum tensor_scalar tiles); the
per-hit charge q enters as a per-partition scalar pointer (exact for
1-bank tiles, middle-of-3 for 3-bank tiles -- f-sorted adjacency makes
that error ~1e-7 of the loss).  Pad slots carry x=100 so their hinge
is exactly zero.  The segment-max/argmax center search runs replicated
on every core over the unique-edge grid; the winning-slot one-hot mask
is PE-transposed and contracted against the bf16 edge-coordinate grid
via ones-vector matmuls.  The member (attractive) correction runs on
the GPSIMD/Pool engine off the dense engines' critical path.  Per-core
accumulator columns are DMA'd out raw and combined on the host (the
unshard step).

Host-side work is strictly index-driven data LAYOUT (slicing, padding,
argsort permutation, reshapes, staging rows of [x|f] into edge grids
by integer edge indices, dtype casts) -- no floating-point arithmetic
on the host beyond the final cross-core reduction of raw partials.
"""
import os
import sys

sys.path.insert(0, "/opt/trn_rl_repo")

import numpy as np
import ml_dtypes

import concourse.bass as bass
import concourse.bacc as bacc
import concourse.tile as tile
from concourse import mybir
from concourse.bass_utils import run_bass_kernel_spmd
import jax as _jax
try:
    os.makedirs(os.path.expanduser("~/.cache/bass_jax_cache"), exist_ok=True)
    _jax.config.update("jax_compilation_cache_dir",
                       os.path.expanduser("~/.cache/bass_jax_cache"))
    _jax.config.update("jax_persistent_cache_min_entry_size_bytes", 0)
    _jax.config.update("jax_persistent_cache_min_compile_time_secs", 0)
except Exception:
    pass
from concourse.masks import make_identity

P = 128
NC = 8
N_HIT = 50000
N_TRUE = 512
D = 8
S_B = 1.0
Q_MIN = 0.5
HPC = N_HIT // NC          # 6250 hits per core
G49 = 49                   # free tiles per partition: 128*49 = 6272
HPAD = P * G49             # padded hits per core
NPAD = HPAD - HPC          # 22 pad slots (x = 100 -> zero hinge)
NT4 = N_TRUE // P          # 4 segment tiles
F32 = mybir.dt.float32
F32R = mybir.dt.float32r
BF16 = mybir.dt.bfloat16
AX = mybir.AxisListType
OP = mybir.AluOpType
ACTF = mybir.ActivationFunctionType

N_WARM = 14                # PE p-state warmup matmuls

# dense-phase tiling: 9 ACT tiles of 3 banks + 22 DVE tiles of 1 bank
A_TILES = [(g, i) for i, g in enumerate(range(0, 27, 3))]          # 9
D_TILES = [(g, 9 + i) for i, g in enumerate(range(27, 49))]        # 22
N_ACC = 31 + 4             # 31 tile accumulators + b2num/nbkg/fcsum/corr
C_B2, C_NB, C_FC, C_CORR = 31, 32, 33, 34


def build_kernel(LF, LC):
    """LF: slots/segment in the replicated center grid.
    LC: slots/segment in the per-core member-correction grid."""
    nc = bacc.Bacc("TRN2", target_bir_lowering=False, debug=False,
                   num_devices=NC)
    vec = nc.vector
    act = nc.scalar
    gps = nc.gpsimd

    # ---------------- I/O ----------------
    gfh_in = nc.dram_tensor("gfh", [P, NT4, 2, LF], F32, kind="ExternalInput")
    hp_in = nc.dram_tensor("hp", [P, D + 3, G49], F32, kind="ExternalInput")
    gxt_in = nc.dram_tensor("gxt", [LF, D, NT4, P], BF16, kind="ExternalInput")
    xt9_in = nc.dram_tensor("xt9", [D + 1, HPAD], F32R, kind="ExternalInput")
    gg_in = nc.dram_tensor("gg", [P, NT4, LC, D + 1], F32, kind="ExternalInput")
    out_all = nc.dram_tensor("out", [P, N_ACC], F32, kind="ExternalOutput")

    with tile.TileContext(nc) as tc:
        with (
            tc.tile_pool(name="sbuf", bufs=1) as sb,
            tc.tile_pool(name="sjunk", bufs=2) as sj,
            tc.tile_pool(name="psum", bufs=2, space="PSUM") as pp,
            tc.tile_pool(name="psa", bufs=2, space="PSUM") as pa,
        ):
            # ------------- loads -------------
            gfh = sb.tile([P, NT4, 2, LF], F32)
            nc.sync.dma_start(out=gfh[:, 0:1, :, :], in_=gfh_in[:, 0:1, :, :])
            nc.sync.dma_start(out=gfh[:, 1:NT4, :, :], in_=gfh_in[:, 1:NT4, :, :])
            hp = sb.tile([P, D + 3, G49], F32)
            nc.sync.dma_start(out=hp[:], in_=hp_in[:])
            gg = sb.tile([P, NT4, LC, D + 1], F32)
            nc.sync.dma_start(out=gg[:], in_=gg_in[:])
            gxt = sb.tile([LF, D, NT4, P], BF16)
            act.dma_start(out=gxt[:], in_=gxt_in[:])
            xt = sb.tile([D + 2, HPAD], F32R)
            act.dma_start(out=xt[0:D + 1, :], in_=xt9_in[:])

            xpm = hp[:, 0:D, :]                  # [P, 8, G49]
            fpm = hp[:, D, :]                    # [P, G49]
            ypm = hp[:, D + 1, :]                # [P, G49]
            nmask = hp[:, D + 2, :]              # [P, G49]

            ident = sb.tile([P, P], F32)
            make_identity(nc, ident[:])
            identb = sb.tile([P, P], BF16)
            make_identity(nc, identb[:])
            cm1 = sb.tile([P, 1], F32)
            gps.memset(cm1[:], -1.0)
            onesb = sb.tile([LF, 1], BF16)
            gps.memset(onesb[:], 1.0)
            acc = sb.tile([P, N_ACC], F32)
            gps.memset(acc[:], 0.0)
            actwarm = sb.tile([P, 1], F32)
            act.activation(out=actwarm[:], in_=cm1[:], func=ACTF.Ln,
                           scale=-1.0)

            # ------------- PE p-state warmup -------------
            for w in range(N_WARM):
                wp = pa.tile([P, 1, 512], F32, space="PSUM", tag="da")
                nc.tensor.matmul(out=wp[:, 0, 0:32], lhsT=ident[:],
                                 rhs=ident[:, 0:32], start=True, stop=True)

            # ------------- center search: per-tile segment max/argmax ------
            fmaxv = sb.tile([P, NT4], F32)
            candp = sb.tile([P, NT4], F32)
            m3 = sb.tile([P, NT4, LF], BF16)
            m3t = sb.tile([LF, NT4, P], BF16)
            for t in range(NT4):
                vec.tensor_reduce(out=fmaxv[:, t:t + 1],
                                  in_=gfh[:, t, 0, :], axis=AX.X, op=OP.max)
            for t in range(NT4):
                eq = sj.tile([P, LF], F32, tag=f"eq{t % 2}")
                vec.tensor_tensor(
                    out=eq[:], in0=gfh[:, t, 0, :],
                    in1=fmaxv[:, t:t + 1].to_broadcast([P, LF]),
                    op=OP.is_equal)
                cnd = sj.tile([P, LF], F32, tag=f"cnd{t % 2}")
                vec.tensor_tensor(out=cnd[:], in0=eq[:], in1=gfh[:, t, 1, :],
                                  op=OP.mult)
                vec.tensor_reduce(out=candp[:, t:t + 1], in_=cnd[:],
                                  axis=AX.X, op=OP.max)
                vec.tensor_tensor(
                    out=m3[:, t, :], in0=gfh[:, t, 1, :],
                    in1=candp[:, t:t + 1].to_broadcast([P, LF]),
                    op=OP.is_equal)
                m3ps = pp.tile([P, 3, 512], F32, space="PSUM", tag="dense")
                m3psb = m3ps[0:LF, 0, :].bitcast(BF16)
                nc.tensor.transpose(out=m3psb[:, 0:P], in_=m3[:, t, :],
                                    identity=identb[:])
                vec.tensor_copy(out=m3t[:, t, :], in_=m3psb[:, 0:P])

            # qc chain (ACT small ops)
            qcb = sj.tile([P, NT4], F32, tag="qcb")
            qcr = sj.tile([P, NT4], F32, tag="qcr")
            qcl = sj.tile([P, NT4], F32, tag="qcl")
            qcs = sj.tile([P, NT4], F32, tag="qcs")
            vec.tensor_scalar(qcb[:], fmaxv[:], -1.0, 1.0, OP.mult, OP.add)
            vec.reciprocal(qcr[:], qcb[:])
            act.activation(out=qcl[:], in_=qcr[:], func=ACTF.Ln,
                           scale=2.0, bias=cm1[:])
            act.activation(out=qcs[:], in_=qcl[:], func=ACTF.Square, scale=0.5)
            qc = sb.tile([P, NT4], F32)
            vec.tensor_scalar(qc[:], qcs[:], Q_MIN, None, OP.add)
            n2qc = sb.tile([P, NT4], F32)
            vec.tensor_scalar(n2qc[:], qc[:], -2.0, None, OP.mult)
            vec.tensor_reduce(out=acc[:, C_FC:C_FC + 1], in_=fmaxv[:],
                              axis=AX.X, op=OP.add)

            # ------------- x^2 row (hit side) -------------
            xsq = sj.tile([P, D, G49], F32, tag="xsq")
            act.activation(out=xsq[:], in_=xpm, func=ACTF.Square)
            x2a = sj.tile([P, 4, G49], F32, tag="x2a")
            vec.tensor_tensor(out=x2a[:], in0=xsq[:, 0:4, :],
                              in1=xsq[:, 4:8, :], op=OP.add)
            x2b = sj.tile([P, 2, G49], F32, tag="x2b")
            vec.tensor_tensor(out=x2b[:], in0=x2a[:, 0:2, :],
                              in1=x2a[:, 2:4, :], op=OP.add)
            x2c = sb.tile([P, G49], F32)
            vec.tensor_tensor(out=x2c[:], in0=x2b[:, 0, :],
                              in1=x2b[:, 1, :], op=OP.add)
            x2ps = pa.tile([P, 1, 512], F32, space="PSUM", tag="da")
            nc.tensor.transpose(out=x2ps[0:G49, 0, 0:P], in_=x2c[:],
                                identity=ident[:])
            x2sb = sb.tile([G49, P], F32R)
            act.activation(out=x2sb[:], in_=x2ps[0:G49, 0, 0:P],
                           func=ACTF.Copy)
            nc.sync.dma_start(out=xt[D + 1:D + 2, :], in_=x2sb[:])

            # ------------- background terms -------------
            bkg = sj.tile([P, G49], F32, tag="bkg")
            vec.tensor_scalar(bkg[:], ypm, -1.0, None, OP.is_equal)
            bf = sj.tile([P, G49], F32, tag="bf")
            gps.tensor_tensor(out=bf[:], in0=bkg[:], in1=fpm, op=OP.mult)
            vec.tensor_reduce(out=acc[:, C_B2:C_B2 + 1], in_=bf[:],
                              axis=AX.X, op=OP.add)
            vec.tensor_reduce(out=acc[:, C_NB:C_NB + 1], in_=bkg[:],
                              axis=AX.X, op=OP.add)

            # ------------- per-hit q (negated, masked) -------------
            qb = sj.tile([P, G49], F32, tag="qb")
            qr = sj.tile([P, G49], F32, tag="qr")
            ql = sj.tile([P, G49], F32, tag="ql")
            qs = sj.tile([P, G49], F32, tag="qs")
            vec.tensor_scalar(qb[:], fpm, -1.0, 1.0, OP.mult, OP.add)
            vec.reciprocal(qr[:], qb[:])
            act.activation(out=ql[:], in_=qr[:], func=ACTF.Ln,
                           scale=2.0, bias=cm1[:])
            act.activation(out=qs[:], in_=ql[:], func=ACTF.Square, scale=0.5)
            negq = sb.tile([P, G49], F32)
            vec.scalar_tensor_tensor(out=negq[:], in0=qs[:], scalar=Q_MIN,
                                     in1=nmask, op0=OP.add, op1=OP.mult)

            # ------------- qe chain (member grid, center-independent) ------
            fe = gg[:, :, :, D]                          # [P, 4, LC]
            eb = sj.tile([P, NT4, LC], F32, tag="eb")
            ec = sj.tile([P, NT4, LC], F32, tag="ec")
            er = sj.tile([P, NT4, LC], F32, tag="er")
            el = sj.tile([P, NT4, LC], F32, tag="el")
            es = sj.tile([P, NT4, LC], F32, tag="es")
            ev = sj.tile([P, NT4, LC], F32, tag="ev")
            vec.tensor_scalar(ec[:], fe, 0.0, -1.0, OP.max, OP.mult)
            vec.tensor_scalar(eb[:], ec[:], 1.0, None, OP.add)
            vec.reciprocal(er[:], eb[:])
            act.activation(out=el[:], in_=er[:], func=ACTF.Ln,
                           scale=2.0, bias=cm1[:])
            act.activation(out=es[:], in_=el[:], func=ACTF.Square, scale=0.5)
            vec.tensor_scalar(ev[:], fe, 0.0, None, OP.is_ge)
            qe = sj.tile([P, NT4, LC], F32, tag="qe")
            vec.scalar_tensor_tensor(out=qe[:], in0=es[:], scalar=Q_MIN,
                                     in1=ev[:], op0=OP.add, op1=OP.mult)

            # ------------- xc selection: mask x grid, contract on PE ------
            xcw = pa.tile([P, 1, 512], F32, space="PSUM", tag="da")
            for d in range(D):
                md = sj.tile([LF, NT4, P], BF16, tag=f"md{d % 4}")
                eng = gps if d < 2 else vec
                eng.tensor_tensor(out=md[:], in0=gxt[:, d, :, :], in1=m3t[:],
                                  op=OP.mult)
                for t in range(NT4):
                    nc.tensor.matmul(out=xcw[:, 0, t * D + d:t * D + d + 1],
                                     lhsT=md[:, t, :], rhs=onesb[:],
                                     start=True, stop=True)
            xcf = sb.tile([P, NT4 * D], F32)
            vec.tensor_copy(out=xcf[:], in_=xcw[:, 0, 0:NT4 * D])
            xc = xcf[:].rearrange("p (t d) -> p t d", d=D)

            # ------------- ctil = [-2qc*xc | qc*(c2-1) | qc] -> cT ------
            csq = sj.tile([P, NT4 * D], F32, tag="csq")
            vec.tensor_tensor(out=csq[:], in0=xcf[:], in1=xcf[:], op=OP.mult)
            c2 = sb.tile([P, NT4], F32)
            vec.tensor_reduce(out=c2[:], in_=csq[:].rearrange(
                "p (t d) -> p t d", d=D), axis=AX.X, op=OP.add)
            ctil = sb.tile([P, NT4, D + 2], F32)
            vec.tensor_tensor(
                out=ctil[:, :, 0:D], in0=xc,
                in1=n2qc[:].to_broadcast([P, NT4, D]), op=OP.mult)
            vec.scalar_tensor_tensor(
                out=ctil[:, :, D:D + 1].rearrange("p t o -> p (t o)"),
                in0=c2[:], scalar=-1.0, in1=qc[:], op0=OP.add, op1=OP.mult)
            vec.tensor_copy(
                out=ctil[:, :, D + 1:D + 2].rearrange("p t o -> p (t o)"),
                in_=qc[:])
            ctps = pa.tile([P, 1, 512], F32, space="PSUM", tag="da")
            for t in range(NT4):
                nc.tensor.transpose(out=ctps[0:D + 2, 0, t * P:(t + 1) * P],
                                    in_=ctil[:, t, :], identity=ident[:])
            cT = sb.tile([D + 2, N_TRUE], F32R)
            act.activation(out=cT[:], in_=ctps[0:D + 2, 0, :], func=ACTF.Copy)

            # ------------- dense phase -------------
            # interleave ACT 3-bank tiles with DVE 1-bank tiles so both
            # engines stay fed; per-partition scalar q from the f-sorted
            # layout (middle column for 3-bank tiles).
            items = []
            ai, di = 0, 0
            pat = [0, 1, 1, 0, 1, 1, 0, 1]   # A : D emission ratio ~9:22
            k = 0
            while ai < len(A_TILES) or di < len(D_TILES):
                want_a = pat[k % len(pat)] == 0
                k += 1
                if want_a and ai < len(A_TILES):
                    items.append(("A",) + A_TILES[ai]); ai += 1
                elif di < len(D_TILES):
                    items.append(("D",) + D_TILES[di]); di += 1
                elif ai < len(A_TILES):
                    items.append(("A",) + A_TILES[ai]); ai += 1

            def dense_item(kind, g0, slot):
                if kind == "A":
                    ps3 = pp.tile([P, 3, 512], F32, space="PSUM", tag="dense")
                    for j in range(3):
                        g = g0 + j
                        nc.tensor.matmul(out=ps3[:, j, :],
                                         lhsT=xt[:, g * P:(g + 1) * P],
                                         rhs=cT[:], start=True, stop=True)
                    act.activation(out=ps3[:], in_=ps3[:], func=ACTF.Relu,
                                   scale=negq[:, g0 + 1:g0 + 2],
                                   accum_out=acc[:, slot:slot + 1])
                else:
                    ps = pa.tile([P, 1, 512], F32, space="PSUM", tag="da")
                    nc.tensor.matmul(out=ps[:, 0, :],
                                     lhsT=xt[:, g0 * P:(g0 + 1) * P],
                                     rhs=cT[:], start=True, stop=True)
                    vec.tensor_scalar(ps[:, 0, :], ps[:, 0, :], 0.0,
                                      negq[:, g0:g0 + 1], OP.min, OP.mult,
                                      accum_out=acc[:, slot:slot + 1])

            for it in items[:10]:
                dense_item(*it)

            # ------------- member correction (Pool; mid-dense) ------
            dif = sj.tile([P, NT4, LC, D], F32, tag="dif")
            gps.tensor_tensor(
                out=dif[:], in0=gg[:, :, :, 0:D],
                in1=xcf[:].rearrange("p (t d) -> p t () d", d=D).to_broadcast(
                    [P, NT4, LC, D]),
                op=OP.subtract)
            dsq = sj.tile([P, NT4, LC, D], F32, tag="dsq")
            gps.tensor_tensor(out=dsq[:], in0=dif[:], in1=dif[:], op=OP.mult)
            ds4 = sj.tile([P, NT4, LC, 4], F32, tag="ds4")
            gps.tensor_tensor(out=ds4[:], in0=dsq[:, :, :, 0:4],
                              in1=dsq[:, :, :, 4:8], op=OP.add)
            ds2 = sj.tile([P, NT4, LC, 2], F32, tag="ds2")
            gps.tensor_tensor(out=ds2[:], in0=ds4[:, :, :, 0:2],
                              in1=ds4[:, :, :, 2:4], op=OP.add)
            dste = sj.tile([P, NT4, LC], F32, tag="dste")
            gps.tensor_tensor(out=dste[:], in0=ds2[:, :, :, 0],
                              in1=ds2[:, :, :, 1], op=OP.add)
            we0 = sj.tile([P, NT4, LC], F32, tag="we0")
            gps.tensor_scalar(we0[:], dste[:], 1.0, 0.0, OP.subtract, OP.min)
            we = sj.tile([P, NT4, LC], F32, tag="we")
            gps.tensor_tensor(out=we[:], in0=we0[:], in1=dste[:], op=OP.add)
            wq = sj.tile([P, NT4, LC], F32, tag="wq")
            gps.tensor_tensor(out=wq[:], in0=we[:],
                              in1=qc[:].to_broadcast([P, NT4, LC]), op=OP.mult)
            wfin = sj.tile([P, NT4, LC], F32, tag="wfin")
            gps.tensor_tensor(out=wfin[:], in0=wq[:], in1=qe[:], op=OP.mult)
            vec.tensor_reduce(out=acc[:, C_CORR:C_CORR + 1], in_=wfin[:],
                              axis=AX.XY, op=OP.add)

            for it in items[10:]:
                dense_item(*it)

            # ------------- output -------------
            nc.sync.dma_start(out=out_all[:], in_=acc[:])

    nc.compile()
    return nc


_CACHE = {}


def _get_kernel(LF, LC):
    key = (LF, LC)
    if key not in _CACHE:
        _CACHE[key] = build_kernel(LF, LC)
    return _CACHE[key]


def _prep(x, f, y, e_h, e_p):
    x = np.asarray(x, np.float32)
    f = np.asarray(f, np.float32)
    y = np.asarray(y).astype(np.int64)
    e_h = np.asarray(e_h).astype(np.int64)
    e_p = np.asarray(e_p).astype(np.int64)

    keys = e_h * N_TRUE + e_p
    ukeys = np.unique(keys)
    uh = (ukeys // N_TRUE).astype(np.int64)
    up = (ukeys % N_TRUE).astype(np.int64)
    order = np.argsort(up, kind="stable")
    uh, up = uh[order], up[order]
    counts = np.bincount(up, minlength=N_TRUE)
    starts = np.zeros(N_TRUE + 1, np.int64)
    np.cumsum(counts, out=starts[1:])
    rank = np.arange(len(up)) - starts[up]

    # full center grid (replicated on every core); seg -> (t = seg//P, p)
    LF = max(4, int(counts.max()))
    ghf = np.full((N_TRUE, LF), -1, np.int64)
    ghf[up, rank] = uh
    valid = ghf >= 0
    gidx = np.clip(ghf, 0, None)
    fg = np.where(valid, f[gidx], -1.0).astype(np.float32)
    hg = np.where(valid, (ghf + 1).astype(np.float32), 0.0).astype(np.float32)
    # gfh: [P, NT4, 2, LF]  (slot 0 = f grid, slot 1 = hit-id+1 grid)
    gfh = np.stack([fg.reshape(NT4, P, LF).transpose(1, 0, 2),
                    hg.reshape(NT4, P, LF).transpose(1, 0, 2)], axis=2)
    # gxt: [LF, D, NT4, P] bf16 (pad 0), edge-slot l on partitions
    gxv = np.where(valid[:, :, None], x[gidx], 0.0)        # [512, LF, D]
    gxt = gxv.reshape(NT4, P, LF, D).transpose(2, 3, 0, 1).astype(
        ml_dtypes.bfloat16)

    # per-core dealt member grids, d innermost: [P, NT4, LC, D+1]
    core = (rank % NC).astype(np.int64)
    slot = (rank // NC).astype(np.int64)
    LC = max(4, int(np.ceil(counts.max() / NC)))
    ghc = np.full((NC, N_TRUE, LC), -1, np.int64)
    ghc[core, up, slot] = uh

    aug = np.concatenate([x, f[:, None]], axis=1)       # [n_hit, 9]
    pad_row = np.zeros(D + 1, np.float32)
    pad_row[D] = -1.0

    in_maps = []
    for c in range(NC):
        g = ghc[c]
        staged = aug[np.clip(g, 0, None)]               # [512, LC, 9]
        staged[g < 0] = pad_row
        gg = np.ascontiguousarray(
            staged.reshape(NT4, P, LC, D + 1).transpose(1, 0, 2, 3))

        sl = slice(c * HPC, (c + 1) * HPC)
        f_core = f[sl]
        # f-sorted layout: s = NPAD pads then ascending-f hits;
        # s -> (p, g) = (s // G49, s % G49)
        sidx = np.argsort(f_core, kind="stable")
        x_loc = np.full((HPAD, D), 100.0, np.float32)   # pads: far away
        f_loc = np.zeros(HPAD, np.float32)
        y_loc = np.zeros(HPAD, np.float32)
        m_loc = np.zeros(HPAD, np.float32)
        x_loc[NPAD:] = x[sl][sidx]
        f_loc[NPAD:] = f_core[sidx]
        y_loc[NPAD:] = y[sl][sidx].astype(np.float32)
        m_loc[NPAD:] = -1.0

        # hp: [P, D+3, G49] = [x (d-outer) | f | y | mask]
        hpk = np.empty((P, D + 3, G49), np.float32)
        hpk[:, 0:D, :] = x_loc.reshape(P, G49, D).transpose(0, 2, 1)
        hpk[:, D, :] = f_loc.reshape(P, G49)
        hpk[:, D + 1, :] = y_loc.reshape(P, G49)
        hpk[:, D + 2, :] = m_loc.reshape(P, G49)

        xt9 = np.empty((D + 1, HPAD), np.float32)
        xt9[:D] = x_loc.reshape(P, G49, D).transpose(2, 1, 0).reshape(D, HPAD)
        xt9[D] = 1.0
        in_maps.append({
            "gfh": np.ascontiguousarray(gfh),
            "hp": hpk,
            "gxt": np.ascontiguousarray(gxt),
            "xt9": np.ascontiguousarray(xt9),
            "gg": gg,
        })
    return in_maps, LF, LC


def kernel(x, f, y, e_h, e_p, trace=False):
    in_maps, LF, LC = _prep(x, f, y, e_h, e_p)
    nc = _get_kernel(LF, LC)
    try:
        res = run_bass_kernel_spmd(nc, in_maps, core_ids=list(range(NC)),
                                   trace=trace)
    except ModuleNotFoundError:
        res = run_bass_kernel_spmd(nc, in_maps, core_ids=list(range(NC)),
                                   trace=False)
    outs = np.stack([res.results[c]["out"].sum(axis=0) for c in range(NC)])
    vtot = outs[:, 0:31].sum() + outs[:, C_CORR].sum()
    b2num = outs[:, C_B2].sum()
    nbkg = outs[:, C_NB].sum()
    fcsum = outs[0, C_FC]
    v = vtot / N_HIT
    b1 = 1.0 - fcsum / N_TRUE
    b2 = S_B * b2num / nbkg
    out = np.array([b1 + b2, v], dtype=np.float32)
    if trace:
        return out, res
    return out
